# revision 3
# baseline (speedup 1.0000x reference)
"""Enframe kernel v3: bf16 stores + all-HWDGE DMA placement.

vs v2: loads move from gpsimd (SWDGE, which stalls when DVE runs 2-port
perf-mode bf16 copies) to the ACT HWDGE ring (nc.scalar), which carries no
other work. All PSUM->SBUF and window copies run on DVE; stores on the SP
HWDGE ring (nc.sync). Deeper a3 prefetch (bufs=18) keeps the SDMA engines
busy through the pipeline ramp.
"""

import numpy as np

import concourse.bacc as bacc
import concourse.bass as bass
import concourse.mybir as mybir
import concourse.tile as tile
from concourse import masks
from concourse.bass_utils import run_bass_kernel_spmd

B, C, S = 16, 4, 160000
FRAME, HOP = 2048, 512
NF = (S - FRAME) // HOP + 1          # 309 frames
NBLK = NF + FRAME // HOP - 1         # 312 blocks of 512 samples actually used
N_CORES = 8
B_PER = B // N_CORES                 # 2 batches per core
F32 = mybir.dt.float32
BF16 = mybir.dt.bfloat16


def build_bass():
    nc = bacc.Bacc(None, target_bir_lowering=False)
    x = nc.dram_tensor("x", [B_PER, C, S], F32, kind="ExternalInput")
    out = nc.dram_tensor("out", [B_PER, C * FRAME, NF], BF16, kind="ExternalOutput")

    with tile.TileContext(nc) as tc:
        with (
            tc.tile_pool(name="singles", bufs=1) as singles,
            tc.tile_pool(name="a", bufs=18) as a_pool,
            tc.tile_pool(name="t2", bufs=6) as t2_pool,
            tc.tile_pool(name="oq", bufs=4) as oq_pool,
            tc.tile_pool(name="ps", bufs=8, space=bass.MemorySpace.PSUM) as ps_pool,
        ):
            ident = singles.tile([128, 128], F32)
            nc.vector.memset(ident[:], 0.0)
            masks.make_identity(nc, ident[:], nomemset=True)

            for b in range(B_PER):
                for c in range(C):
                    slab_off = (b * C + c) * S
                    # T2[p, i, j] = X2[j, 4p + i], bf16
                    t2 = t2_pool.tile([128, 4, NBLK], BF16)
                    for jt in range(3):
                        pj = 128 if jt < 2 else NBLK - 256  # 128, 128, 56
                        a3 = a_pool.tile([128, HOP], F32)
                        src = bass.AP(x, slab_off + jt * 128 * HOP,
                                      [[HOP, pj], [1, HOP]])
                        ld = nc.sync if (b == 0 and c == 0 and jt == 0) else nc.scalar
                        ld.dma_start(out=a3[:pj], in_=src)
                        for i in range(4):
                            pst = ps_pool.tile([128, 128], F32)
                            nc.tensor.transpose(
                                pst[:, :pj], a3[:pj, i::4], ident[:pj, :pj]
                            )
                            nc.vector.tensor_copy(
                                out=t2[:, i, jt * 128 : jt * 128 + pj],
                                in_=pst[:, :pj],
                            )
                    # oall[p, q, i, f] = t2[p, i, q + f]; one contiguous
                    # 1,265,664B HBM range per slab stored in a single DMA.
                    oall = oq_pool.tile([128, 4, 4, NF], BF16)
                    for q in range(4):
                        nc.vector.tensor_copy(out=oall[:, q], in_=t2[:, :, q : q + NF])
                    dst = bass.AP(
                        out,
                        (b * C * FRAME + c * FRAME) * NF,
                        [[4 * NF, 128], [HOP * NF, 4], [NF, 4], [1, NF]],
                    )
                    nc.sync.dma_start(out=dst, in_=oall[:])
    nc.finalize()
    return nc


_NC_CACHE = None


def kernel(x: np.ndarray) -> np.ndarray:
    global _NC_CACHE
    if _NC_CACHE is None:
        _NC_CACHE = build_bass()
    nc = _NC_CACHE
    in_maps = [
        {"x": np.ascontiguousarray(x[i * B_PER : (i + 1) * B_PER])}
        for i in range(N_CORES)
    ]
    res = run_bass_kernel_spmd(nc, in_maps, list(range(N_CORES)))
    return np.concatenate(
        [np.asarray(r["out"]).astype(np.float32) for r in res.results], axis=0
    )
